# revision 33
# baseline (speedup 1.0000x reference)
"""Trainium2 Bass kernel for nn_MessageFunctionForEvent (GNN message function).

Math: the reference is
    em  = W_e2m @ e_wv[b] + b_e2m          (per-node Linear on edge features)
    nw  = W_n2m @ h_w[b]  + b_n2m          (per-node Linear on node features)
    nv  = W_n2m @ h_v[b]  + b_n2m          (node-level, no n axis)
    msg = Wa @ em + Wb @ nw + (Wc @ nv + b_resize)[:, None]
which collapses (precomposing the tiny 128x128 weights on host) to
    msg[b, :, n] = A @ e_wv[b, :, n] + Bm @ h_w[b, :, n] + c[b]
with A = Wa@W_e2m, Bm = Wb@W_n2m, c[b] = Wa@b_e2m + Wb@b_n2m + Wc@nv[b] + b_resize.

The kernel is pure HBM/DMA-bandwidth bound (target_regime=memory): per core
it must read the e/h shards and write the msg shard. To halve that traffic
the host casts e_wv/h_w to bf16 (inputs are O(1) randn; bf16 rounding is
~0.1% rel err, far under the 2e-2 gate), the device matmuls in bf16 with
fp32 PSUM accumulate, and the drains write a bf16 output tile that the
host upcasts back to fp32.

Device pipeline (per core = 2 batches):
  - e and h chunks stream HBM->SBUF on the sync HWDGE ring with a 7-deep
    chunk prefetch (35k columns of lookahead) so the DMA queues never go
    empty when the PE hiccups (HAM throttle); the queued backlog keeps
    the 16 SDMA engines at their ~26GB/s streaming rate (~404 GB/s
    aggregate, the practical per-core ceiling).
  - Per 500-col PSUM tile: accumulated A- then B-matmul (bf16, fp32 PSUM),
    then the drain (bias add + fp32->bf16 cast) alternates between
    VectorE tensor_scalar_add and ACT activation so neither engine is
    the bottleneck.
  - Whole-chunk output flushes on the scalar HWDGE ring, issued directly
    after the chunk's own last ACT drain so the strict-FIFO ACT queue
    never waits at a drain-gated DMA issue (pairing flushes across
    chunks head-of-line-blocks for ~15us; per-chunk flushes don't).
Sharding: batch axis (16 batches -> 2 per core), zero host re-layout
beyond the bf16 cast.
"""

import sys

import ml_dtypes
import numpy as np

try:
    from concourse import bacc, mybir
except ImportError:  # bare environment: fall back to the in-container repo
    sys.path.append("/opt/trn_rl_repo")
    from concourse import bacc, mybir
import concourse.tile as tile
from concourse.bass_utils import run_bass_kernel_spmd

B, F, N = 16, 128, 20000
NCORES = 8
BPC = B // NCORES          # batches per core
CH = 5000                  # columns per DMA chunk (1.28 MB bf16 transfers)
NT = 500                   # columns per matmul (fits one 2KB fp32 PSUM bank)

BF16 = ml_dtypes.bfloat16

_cached_nc = None


def _chunks_for(b):
    # chunk schedule per (core-local) batch: big streaming chunks, tapered
    # at the very end so the pipeline drains with small PE/DMA quanta
    # instead of one full-size chunk of latency.
    if b < BPC - 1:
        return [CH] * (N // CH)
    return [CH, CH, CH, 2000, 1000, 1000, 500, 500]


def _build():
    global _cached_nc
    if _cached_nc is not None:
        return _cached_nc
    f32 = mybir.dt.float32
    bf16 = mybir.dt.bfloat16
    nc = bacc.Bacc("TRN2", target_bir_lowering=False, debug=False,
                   num_devices=NCORES)
    e_d = nc.dram_tensor("e_wv", (BPC, F, N), bf16, kind="ExternalInput").ap()
    h_d = nc.dram_tensor("h_w", (BPC, F, N), bf16, kind="ExternalInput").ap()
    at_d = nc.dram_tensor("at", (F, F), bf16, kind="ExternalInput").ap()
    bt_d = nc.dram_tensor("bt", (F, F), bf16, kind="ExternalInput").ap()
    c_d = nc.dram_tensor("c", (F, BPC), f32, kind="ExternalInput").ap()
    o_d = nc.dram_tensor("msg", (BPC, F, N), bf16, kind="ExternalOutput").ap()

    with tile.TileContext(nc) as tc:
        with tc.tile_pool(name="w", bufs=1) as wp, \
             tc.tile_pool(name="eh", bufs=7) as ehp, \
             tc.tile_pool(name="out", bufs=4) as opp, \
             tc.tile_pool(name="ps", bufs=8, space="PSUM") as psp:
            at_t = wp.tile([F, F], bf16)
            nc.gpsimd.dma_start(at_t[:], at_d[:])
            bt_t = wp.tile([F, F], bf16)
            nc.gpsimd.dma_start(bt_t[:], bt_d[:])
            c_t = wp.tile([F, BPC], f32)
            nc.gpsimd.dma_start(c_t[:], c_d[:])
            for b in range(BPC):
                n0 = 0
                for cj, cs in enumerate(_chunks_for(b)):
                    sl = slice(n0, n0 + cs)
                    e_t = ehp.tile([F, cs], bf16, tag="e")
                    h_t = ehp.tile([F, cs], bf16, tag="h")
                    o_t = opp.tile([F, cs], bf16, tag="o")
                    nc.sync.dma_start(e_t[:], e_d[b, :, sl])
                    nc.sync.dma_start(h_t[:], h_d[b, :, sl])
                    # taper chunks split into >=2 PSUM tiles so their
                    # drain runs on VectorE and ACT in parallel (the last
                    # chunk's drain sits directly on the kernel tail)
                    nk = max(2, cs // NT) if cs >= 500 else 1
                    nt = cs // nk
                    ps_ts = []
                    for k in range(nk):
                        ksl = slice(k * nt, (k + 1) * nt)
                        ps_t = psp.tile([F, nt], f32, tag="ps")
                        ps_ts.append(ps_t)
                        nc.tensor.matmul(ps_t[:], at_t[:], e_t[:, ksl],
                                         start=True, stop=False)
                    for k in range(nk):
                        ksl = slice(k * nt, (k + 1) * nt)
                        nc.tensor.matmul(ps_ts[k][:], bt_t[:], h_t[:, ksl],
                                         start=False, stop=True)
                        # split the PSUM drain (bias add + fp32->bf16 cast)
                        # across VectorE and the ACT engine
                        if k % 2 == 0:
                            nc.vector.tensor_scalar_add(o_t[:, ksl],
                                                        ps_ts[k][:],
                                                        c_t[:, b:b + 1])
                        else:
                            nc.scalar.activation(
                                o_t[:, ksl], ps_ts[k][:],
                                mybir.ActivationFunctionType.Identity,
                                bias=c_t[:, b:b + 1])
                    # whole-chunk flush right after this chunk's own last
                    # drain: the ACT FIFO reaches the issue with its wait
                    # already satisfied, so no head-of-line blocking
                    nc.scalar.dma_start(o_d[b, :, sl], o_t[:])
                    n0 += cs
    nc.finalize()
    _cached_nc = nc
    return nc


def _prepare_in_maps(h_w, h_v, e_wv, W_e2m, b_e2m, W_n2m, b_n2m,
                     W_resize, b_resize):
    f64 = np.float64
    M = F
    Wa = W_resize[:, :M].astype(f64)
    Wb = W_resize[:, M:2 * M].astype(f64)
    Wc = W_resize[:, 2 * M:].astype(f64)
    A = Wa @ W_e2m.astype(f64)
    Bm = Wb @ W_n2m.astype(f64)
    nv = h_v.astype(f64) @ W_n2m.astype(f64).T + b_n2m.astype(f64)
    c = (Wa @ b_e2m.astype(f64) + Wb @ b_n2m.astype(f64)
         + nv @ Wc.T + b_resize.astype(f64))          # [B, M]
    AT = np.ascontiguousarray(A.T).astype(BF16)
    BT = np.ascontiguousarray(Bm.T).astype(BF16)
    cT = np.ascontiguousarray(c.T).astype(np.float32)  # [M, B]

    e_bf = e_wv.astype(BF16)
    h_bf = h_w.astype(BF16)

    in_maps = []
    for cid in range(NCORES):
        bs = slice(cid * BPC, (cid + 1) * BPC)
        in_maps.append({
            "e_wv": e_bf[bs],
            "h_w": h_bf[bs],
            "at": AT,
            "bt": BT,
            "c": np.ascontiguousarray(cT[:, bs]),
        })
    return in_maps


def kernel(**inputs):
    args = {k: np.asarray(inputs[k], dtype=np.float32)
            for k in ("h_w", "h_v", "e_wv", "W_e2m", "b_e2m", "W_n2m",
                      "b_n2m", "W_resize", "b_resize")}
    in_maps = _prepare_in_maps(**args)
    nc = _build()
    res = run_bass_kernel_spmd(nc, in_maps, core_ids=list(range(NCORES)))
    return np.concatenate(
        [r["msg"].astype(np.float32) for r in res.results], axis=0)
